# revision 44
# baseline (speedup 1.0000x reference)
"""GCN inference kernel (y = D^-1/2 A D^-1/2 (x @ W.T)) on 8 Trainium2 NeuronCores.

Strategy (full inputs in, full output out; sharded internally):
  - Destination nodes are sharded across the 8 cores (12500 dsts each);
    edges are owned by the core that owns their dst, so the segment-sum is
    core-local (per the sharding hint).
  - Phase A (sharded): each core computes the scaled projection table
    h~[n] = dinv[n] * (x[n] @ W.T) for its 12800-node shard with PE matmuls
    (bf16), writing bf16 rows padded to 256B. The table is laid out
    quarter-major so four per-quarter AllGathers pipeline with the shard
    compute and with phase B (bucket b's gathers start as soon as
    collective b lands).
  - Phase B (per core): SWDGE dma_gather streams h~[src] rows (256B each,
    only cols 0:64 carry data) for the core's dst-sorted edge list into
    SBUF; a one-hot selection matrix B (built on DVE in bf16 from dst-local
    ids vs an iota row) turns the segment-sum into PE matmuls accumulated
    in PSUM per 128-dst tile; a final per-dst dinv scale lands y.
  - One gather call per (superwindow, bucket, dst-tile) cell; the per-core
    number of REAL edges in the call is loaded into a GPSIMD register at
    runtime (num_idxs_reg) so the schedule's padding slots cost no gather
    descriptors. Pad slots keep dstl=-1 so their one-hot column is zero.
  - All data-dependent structure (edge sort, padding, gather indices,
    one-hot ids, uniform per-core slice schedule) is prepared host-side in
    numpy; the device program is identical on all 8 cores (SPMD), only the
    per-core input arrays differ.
"""

import contextlib
import math
from dataclasses import dataclass, field

import ml_dtypes
import numpy as np

import concourse.bacc as bacc
import concourse.bass as bass
import concourse.mybir as mybir
import concourse.tile as tile
from concourse import library_config
from concourse.bass_utils import run_bass_kernel_spmd

P = 128  # SBUF partitions
FIN = 128
FOUT = 64
TROW = 2 * FOUT  # table row: 64 bf16 data + 64 bf16 pad = 256B


@dataclass
class Prm:
    N: int = 100000  # nodes
    C: int = 8  # cores
    WG: int = 640  # nodes per phase-A write group
    GQ: int = 5  # write groups per quarter (pipelined collective unit)
    SWD: int = 512  # dst nodes per superwindow (TPSW * P)
    NBUFS: int = 24  # gather/one-hot pool depth (in calls)
    J: int = field(init=False)
    NS: int = field(init=False)  # dst shard size per core
    N2: int = field(init=False)  # padded node count (multiple of C*WG)
    NG: int = field(init=False)  # phase-A write groups
    NGpc: int = field(init=False)  # phase-A write groups per core
    NBK: int = field(init=False)  # gather buckets (= collective quarters)
    SHN: int = field(init=False)  # nodes per phase-A shard
    QN: int = field(init=False)  # nodes per (core, quarter)
    BKCAP: int = field(init=False)  # table rows per gather bucket
    TBLR: int = field(init=False)  # total table rows
    TPSW: int = field(init=False)  # dst tiles per superwindow
    NSW: int = field(init=False)  # superwindows per core

    def __post_init__(self):
        assert self.WG % P == 0
        assert self.SWD % P == 0
        assert self.N % self.C == 0
        self.J = self.WG // P
        self.NS = self.N // self.C
        blk = self.C * self.WG
        self.N2 = ((self.N + blk - 1) // blk) * blk
        self.NG = self.N2 // self.WG
        self.NGpc = self.NG // self.C
        assert self.NGpc % self.GQ == 0
        self.NBK = self.NGpc // self.GQ
        self.SHN = self.N2 // self.C
        self.QN = self.GQ * self.WG
        self.BKCAP = self.C * self.QN
        assert self.BKCAP <= 32767
        self.TBLR = self.N2
        self.TPSW = self.SWD // P
        self.NSW = (self.NS + self.SWD - 1) // self.SWD


def _rmap(prm, n):
    """node id -> table row, quarter-major layout: bucket k holds quarter k
    of every core's shard (so per-quarter AllGathers land contiguously)."""
    c = n // prm.SHN
    i2 = n % prm.SHN
    k = i2 // prm.QN
    i = i2 % prm.QN
    wrap = prm.WG * (i // prm.WG) + prm.J * (i % P) + (i % prm.WG) // P
    return k * prm.BKCAP + c * prm.QN + wrap


def _wrap_idx(vals16):
    """[K] int16 (K % 128 == 0) -> [128, K//16] wrapped+replicated layout."""
    k = vals16.shape[0]
    w16 = vals16.reshape(k // 16, 16).T  # [16, K/16]
    return np.tile(w16, (8, 1))  # [128, K/16]


@dataclass
class CallMeta:
    sw: int
    bk: int
    t: int
    S: int  # slices in this call (one dma_gather per call)
    icol: int  # column offset into gidx array (8 * slice offset)
    scol: int  # column offset into dstl array (slice offset)


def _schedule(prm, n_sl_u):
    """Uniform (core-independent) schedule: one gather call per non-empty
    (sw, bk, t) cell. Matmuls are emitted bucket-major per sw so PE starts
    as soon as bucket 0's gather lands; each dst-tile t accumulates into its
    own PSUM tensor across buckets (start on its first mm, stop on last).

    Returns (calls, mms_by_sw, icol_total, scol_total).
    mms_by_sw[sw] = list of (bk, t, sl, start, stop); lhsT/rhs come from
    call (sw, bk, t) local slice sl.
    """
    calls = []
    mms_by_sw = []
    icol = 0
    scol = 0
    for sw in range(prm.NSW):
        tot = [
            sum(int(n_sl_u[sw][bk][t]) for bk in range(prm.NBK))
            for t in range(prm.TPSW)
        ]
        seen = [0] * prm.TPSW
        mms = []
        for bk in range(prm.NBK):
            for t in range(prm.TPSW):
                S = int(n_sl_u[sw][bk][t])
                if S == 0:
                    continue
                calls.append(CallMeta(sw, bk, t, S, icol, scol))
                icol += 8 * S
                scol += S
                for sl in range(S):
                    mms.append(
                        (bk, t, sl, seen[t] == 0, seen[t] == tot[t] - 1)
                    )
                    seen[t] += 1
        mms_by_sw.append(mms)
    return calls, mms_by_sw, icol, scol


def _host_prep(x, edge_index, W, prm):
    N, C, NS = prm.N, prm.C, prm.NS
    src = np.asarray(edge_index[0], dtype=np.int64).astype(np.int32)
    dst = np.asarray(edge_index[1], dtype=np.int64).astype(np.int32)
    x = np.asarray(x, dtype=np.float32)
    W = np.asarray(W, dtype=np.float32)

    deg = np.bincount(dst, minlength=N).astype(np.float64)
    dinv = np.where(deg > 0, 1.0 / np.sqrt(np.maximum(deg, 1.0)), 0.0).astype(
        np.float32
    )

    # gather-order node map
    r_of = _rmap(prm, np.arange(N, dtype=np.int64)).astype(np.int64)
    bk_of = (r_of // prm.BKCAP).astype(np.int32)
    rel_of = (r_of % prm.BKCAP).astype(np.int16)

    # per-edge attributes
    core_e = dst // NS
    edl = dst - core_e * NS
    sw_e = edl // prm.SWD
    t_e = (edl % prm.SWD) // P
    q_e = (edl % P).astype(np.float32)
    bk_e = bk_of[src]
    rel_e = rel_of[src]

    # per-core cell structure; edges sorted by table row within each cell
    # (HBM page locality for the gather stream)
    ncell = prm.NSW * prm.NBK * prm.TPSW
    counts = np.zeros((C, ncell), dtype=np.int64)
    percore = []
    for c in range(C):
        m = core_e == c
        order = np.lexsort((rel_e[m], t_e[m], bk_e[m], sw_e[m]))
        cell = (sw_e[m] * prm.NBK + bk_e[m]) * prm.TPSW + t_e[m]
        counts[c] = np.bincount(cell, minlength=ncell)
        percore.append(
            {
                "rel": rel_e[m][order],
                "q": q_e[m][order],
                "cell": cell[order],
            }
        )

    # uniform slice counts; ensure every in-range (sw, t) has >= 1 slice
    # somewhere so its PSUM accumulation group opens and closes
    n_sl_u = np.zeros((prm.NSW, prm.NBK, prm.TPSW), dtype=np.int64)
    cmax = counts.max(axis=0).reshape(prm.NSW, prm.NBK, prm.TPSW)
    n_sl_u[:] = (cmax + P - 1) // P
    for sw in range(prm.NSW):
        ntile = min(prm.TPSW, max(0, -(-(NS - sw * prm.SWD) // P)))
        for t in range(ntile):
            if n_sl_u[sw, :, t].sum() == 0:
                n_sl_u[sw, 0, t] = 1

    calls, mms_by_sw, icols, scols = _schedule(prm, n_sl_u)

    # slot offset (in slices) of each cell in the uniform stream
    cell_sl = n_sl_u.reshape(ncell)
    cell_off = np.zeros(ncell, dtype=np.int64)
    np.cumsum(cell_sl[:-1], out=cell_off[1:])
    S_total = int(cell_sl.sum())

    # fill per-core gather-index / dst-local / valid-count arrays
    gidx_all = np.zeros((C, P, icols), dtype=np.int16)
    dstl_all = np.full((C, P, scols), -1.0, dtype=ml_dtypes.bfloat16)
    nval_all = np.zeros((C, len(calls)), dtype=np.int32)
    cell_to_call = {}
    for ci, cm in enumerate(calls):
        cell_to_call[(cm.sw * prm.NBK + cm.bk) * prm.TPSW + cm.t] = ci
    for c in range(C):
        pc = percore[c]
        ne = pc["cell"].shape[0]
        cc = counts[c]
        starts = np.zeros(ncell, dtype=np.int64)
        np.cumsum(cc[:-1], out=starts[1:])
        rank = np.arange(ne, dtype=np.int64) - starts[pc["cell"]]
        pos = cell_off[pc["cell"]] * P + rank  # global slot position
        vals = np.full(S_total * P, -1, dtype=np.int16)
        dvals = np.full(S_total * P, -1.0, dtype=np.float32)
        vals[pos] = pc["rel"]
        dvals[pos] = pc["q"]
        for ci, cm in enumerate(calls):
            sl0 = cm.scol
            seg = vals[sl0 * P : (sl0 + cm.S) * P].copy()
            nv = int(cc[(cm.sw * prm.NBK + cm.bk) * prm.TPSW + cm.t])
            assert nv <= cm.S * P
            if nv == 0:
                # the gather ucode (and sim) need >= 1 valid index
                seg[0] = 0
                nv = 1
            nval_all[c, ci] = nv
            gidx_all[c, :, cm.icol : cm.icol + 8 * cm.S] = _wrap_idx(seg)
            dstl_all[c, :, cm.scol : cm.scol + cm.S] = (
                dvals[sl0 * P : (sl0 + cm.S) * P].reshape(cm.S, P).T
            )

    # phase-A inputs
    xT = np.zeros((FIN, prm.N2), dtype=ml_dtypes.bfloat16)
    xT[:, :N] = x.T.astype(ml_dtypes.bfloat16)
    WT = np.ascontiguousarray(W.T).astype(ml_dtypes.bfloat16)  # [FIN, FOUT]
    dinvA = np.zeros((P, prm.NG * prm.J), dtype=np.float32)
    n_idx = np.arange(prm.N2)
    g_i, j_i, p_i = n_idx // prm.WG, (n_idx % prm.WG) // P, n_idx % P
    dpad = np.zeros(prm.N2, dtype=np.float32)
    dpad[:N] = dinv
    dinvA[p_i, g_i * prm.J + j_i] = dpad
    iota = np.broadcast_to(
        np.arange(P, dtype=ml_dtypes.bfloat16)[None, :], (P, P)
    ).copy()
    dinvD = np.zeros((C, P, prm.NSW * prm.TPSW), dtype=np.float32)
    w_idx = np.arange(prm.NSW * prm.TPSW)
    for c in range(C):
        node = c * NS + w_idx[:, None] * P + np.arange(P)[None, :]
        ok = node < (c + 1) * NS
        dv = np.where(ok, dinv[np.minimum(node, N - 1)], 0.0)
        dinvD[c][np.arange(P)[None, :], w_idx[:, None]] = dv

    inputs = []
    for c in range(C):
        inputs.append(
            {
                "xT": xT,  # phase A replicated: full table on every core
                "WT": WT,
                "dinvA": dinvA,
                "iota": iota,
                "dinvD": dinvD[c],
                "gidx": gidx_all[c],
                "dstl": dstl_all[c],
                "nval": nval_all[c : c + 1],  # [1, ncalls]
            }
        )
    return inputs, calls, mms_by_sw


def _split_sync_waits(nc):
    """This env's walrus rejects >1 sync wait on some opcodes; keep 1 wait
    per instruction, moving extras onto preceding same-engine NOPs."""
    for bb in nc.main_func.blocks:
        insts = bb.instructions
        i = 0
        while i < len(insts):
            ins = insts[i]
            si = ins.sync_info
            if si is not None and si.on_wait is not None and len(si.on_wait) > 1:
                waits = list(si.on_wait)
                keep, extra = waits[-1:], waits[:-1]
                k = 0
                while extra:
                    chunk, extra = extra[:1], extra[1:]
                    nop = mybir.InstNoOp(name=f"{ins.name}-ws{k}", ins=[], outs=[])
                    nop.engine = ins.engine
                    nop.sync_info = mybir.SyncInfo(on_wait=chunk, on_update=[])
                    nc.register_instruction(nop)
                    insts.insert(i, nop)
                    i += 1
                    k += 1
                ins.sync_info = mybir.SyncInfo(
                    on_wait=keep, on_update=list(si.on_update or [])
                )
            i += 1


def _build_program(prm, calls, mms_by_sw, icols, scols, mode="full"):
    f32 = mybir.dt.float32
    bf16 = mybir.dt.bfloat16
    i32 = mybir.dt.int32
    nc = bacc.Bacc("TRN2", num_swdge_queues=4)

    ncalls = max(1, len(calls))
    xT = nc.declare_dram_parameter("xT", [FIN, prm.N2], bf16, isOutput=False)
    WT = nc.declare_dram_parameter("WT", [FIN, FOUT], bf16, isOutput=False)
    dinvA = nc.declare_dram_parameter(
        "dinvA", [P, prm.NG * prm.J], f32, isOutput=False
    )
    iota = nc.declare_dram_parameter("iota", [P, P], bf16, isOutput=False)
    dinvD = nc.declare_dram_parameter(
        "dinvD", [P, prm.NSW * prm.TPSW], f32, isOutput=False
    )
    gidx = nc.declare_dram_parameter("gidx", [P, icols], mybir.dt.int16, isOutput=False)
    dstl = nc.declare_dram_parameter("dstl", [P, scols], bf16, isOutput=False)
    nval = nc.declare_dram_parameter("nval", [1, ncalls], i32, isOutput=False)
    y = nc.declare_dram_parameter("y", [prm.NS, FOUT], f32, isOutput=True)
    with tile.TileContext(nc) as tc:
        # phase A is REPLICATED: every core computes the full table itself
        # (26MB xT stream overlaps early gathers; no collective rendezvous).
        # Tile-tracked DRAM tiles so bucket k's gathers automatically wait
        # for its 40 table-write DMAs (Tile doesn't track raw dram_tensors).
        TBL = [
            tc.tile(
                [prm.BKCAP, TROW], bf16, space="DRAM", name=f"tbl{k}", tag=f"tbl{k}"
            )[0]
            for k in range(prm.NBK)
        ]
        with tc.tile_pool(name="const", bufs=1) as cpool:
            wt_sb = cpool.tile([FIN, FOUT], bf16, tag="wt")
            nc.sync.dma_start(out=wt_sb[:], in_=WT[:])
            dinvA_sb = cpool.tile([P, prm.NG * prm.J], f32, tag="da")
            nc.sync.dma_start(out=dinvA_sb[:], in_=dinvA[:])
            iota_sb = cpool.tile([P, P], bf16, tag="io")
            nc.sync.dma_start(out=iota_sb[:], in_=iota[:])
            dinvD_sb = cpool.tile([P, prm.NSW * prm.TPSW], f32, tag="dd")
            nc.sync.dma_start(out=dinvD_sb[:], in_=dinvD[:])
            nval_sb = cpool.tile([1, ncalls], i32, tag="nv")
            nc.sync.dma_start(out=nval_sb[:], in_=nval[:])

            # All pools are opened together: closing phase A's pools before
            # opening phase B's would recycle their SBUF into the new pools,
            # and Tile then serializes all of phase B behind the last
            # phase-A reader of that space.
            _stk = contextlib.ExitStack()
            pa = _stk.enter_context(tc.tile_pool(name="pa", bufs=4))
            psa = _stk.enter_context(tc.tile_pool(name="psa", bufs=2, space="PSUM"))
            pidx = _stk.enter_context(tc.tile_pool(name="pidx", bufs=prm.NBUFS))
            pg = _stk.enter_context(tc.tile_pool(name="pg", bufs=prm.NBUFS))
            pb = _stk.enter_context(tc.tile_pool(name="pb", bufs=prm.NBUFS))
            py = _stk.enter_context(tc.tile_pool(name="py", bufs=3))
            psb = _stk.enter_context(tc.tile_pool(name="psb", bufs=1, space="PSUM"))

            # ------- Phase A: full table per core, bucket-major -----------
            ng_emitted = 0
            for k in range(prm.NBK):
                for c2 in range(prm.C):
                    for gq in range(prm.GQ):
                        # nodes c2*SHN + k*QN + gq*WG + [0, WG)
                        n0 = c2 * prm.SHN + k * prm.QN + gq * prm.WG
                        g = n0 // prm.WG
                        xt = pa.tile([P, prm.WG], bf16, tag="xt")
                        nc.sync.dma_start(
                            out=xt[:], in_=xT[:, n0 : n0 + prm.WG]
                        )
                        hps = psa.tile([P, prm.J * FOUT], f32, tag="hps")
                        for j in range(prm.J):
                            nc.tensor.matmul(
                                out=hps[:, j * FOUT : (j + 1) * FOUT],
                                lhsT=xt[:, j * P : (j + 1) * P],
                                rhs=wt_sb[:],
                                start=True,
                                stop=True,
                            )
                        tsb = pa.tile([P, prm.J, TROW], bf16, tag="tsb")
                        if ng_emitted < 4:  # zero pad cols once per buffer
                            nc.vector.memset(tsb[:], 0.0)
                        ng_emitted += 1
                        nc.vector.tensor_tensor(
                            out=tsb[:, :, :FOUT],
                            in0=hps[:].rearrange("p (j f) -> p j f", f=FOUT),
                            in1=dinvA_sb[:, g * prm.J : (g + 1) * prm.J][
                                :, :, None
                            ].to_broadcast([P, prm.J, FOUT]),
                            op=mybir.AluOpType.mult,
                        )
                        base = c2 * prm.QN + gq * prm.WG
                        nc.sync.dma_start(
                            out=TBL[k][base : base + prm.WG, :].rearrange(
                                "(p j) f -> p j f", j=prm.J
                            ),
                            in_=tsb[:],
                        )

            # ---------------- Phase B: gather + segment-sum ----------------
            if mode == "phaseA":
                with tc.tile_pool(name="pz", bufs=1) as pz:
                    zy = pz.tile([P, FOUT], f32, tag="zy")
                    nc.vector.memset(zy[:], 0.0)
                    for r0 in range(0, prm.NS, P):
                        rt = min(P, prm.NS - r0)
                        nc.sync.dma_start(out=y[r0 : r0 + rt, :], in_=zy[:rt, :])
                calls = []
            S_MAX = max((cm.S for cm in calls), default=1)
            calls_by_swbk = {}
            for ci, cm in enumerate(calls):
                calls_by_swbk.setdefault((cm.sw, cm.bk), []).append((ci, cm))
            nreg = nc.gpsimd.alloc_register("nvreg")
            qctr = [0]
            emitted = [0]
            bk_waited = set()

            def emit_calls(sw, bk, tiles):
                for ci, cm in calls_by_swbk.get((sw, bk), []):
                    S = cm.S
                    idx_t = pidx.tile([P, 8 * S_MAX], mybir.dt.int16, tag="idx")
                    nc.scalar.dma_start(
                        out=idx_t[:, : 8 * S],
                        in_=gidx[:, cm.icol : cm.icol + 8 * S],
                    )
                    dst_t = pidx.tile([P, S_MAX], bf16, tag="dst")
                    nc.scalar.dma_start(
                        out=dst_t[:, :S], in_=dstl[:, cm.scol : cm.scol + S]
                    )
                    g_t = pg.tile([P, S_MAX, TROW], bf16, tag="g")
                    # zero each pool buffer on its first use: slots skipped by
                    # the runtime-count gather must hold finite data (0*NaN
                    # would poison the PE accumulation)
                    if emitted[0] < prm.NBUFS:
                        nc.vector.memset(g_t[:], 0.0)
                    emitted[0] += 1
                    if mode == "nogather":
                        nc.vector.memset(g_t[:, :S, :], 0.0)
                    else:
                        nc.gpsimd.reg_load(nreg, nval_sb[0:1, ci : ci + 1])
                        nc.gpsimd.dma_gather(
                            out_ap=g_t[:, :S, :],
                            in_ap=TBL[cm.bk][:],
                            idxs_ap=idx_t[:, : 8 * S],
                            num_idxs=S * P,
                            num_idxs_reg=nreg,
                            elem_size=TROW,
                            single_packet=False,
                            queue_num=qctr[0] % 4,
                        )
                        qctr[0] += 1
                    b_t = pb.tile([P, S_MAX, P], bf16, tag="b")
                    nc.vector.tensor_tensor(
                        out=b_t[:, :S, :],
                        in0=dst_t[:, :S][:, :, None].to_broadcast([P, S, P]),
                        in1=iota_sb[:, None, :].to_broadcast([P, S, P]),
                        op=mybir.AluOpType.is_equal,
                    )
                    tiles[(cm.bk, cm.t)] = (g_t, b_t)

            def emit_mms(sw, tiles):
                # one PSUM bank per dst tile: start=True clears the whole
                # bank, so accumulation groups must never share one
                psum_t = [
                    psb.tile([P, FOUT], f32, tag=f"acc{t}", name=f"acc{t}")
                    for t in range(prm.TPSW)
                ]
                for bk, t, sl, st, sp in mms_by_sw[sw]:
                    g_t, b_t = tiles[(bk, t)]
                    nc.tensor.matmul(
                        out=psum_t[t][:],
                        lhsT=b_t[:, sl, :],
                        rhs=g_t[:, sl, :FOUT],
                        start=st,
                        stop=sp,
                    )
                # scale by dinv[dst] (on the otherwise-idle Scalar engine,
                # keeping DVE free for the next superwindow's B-builds)
                rows_sw = min(prm.SWD, prm.NS - sw * prm.SWD)
                nt = (rows_sw + P - 1) // P  # valid dst tiles this sw
                ysb = py.tile([P, prm.TPSW, FOUT], f32, tag="ysb")
                for t in range(nt):
                    w = sw * prm.TPSW + t
                    nc.scalar.activation(
                        out=ysb[:, t, :],
                        in_=psum_t[t][:],
                        func=mybir.ActivationFunctionType.Copy,
                        scale=dinvD_sb[:, w : w + 1],
                    )
                for t in range(nt):
                    rt = min(P, rows_sw - t * P)
                    r0 = sw * prm.SWD + t * P
                    nc.sync.dma_start(out=y[r0 : r0 + rt, :], in_=ysb[:rt, t, :])

            for sw in range(prm.NSW):
                tiles = {}
                for bk in range(prm.NBK):
                    emit_calls(sw, bk, tiles)
                emit_mms(sw, tiles)
            _stk.close()

    nc.compile()
    _split_sync_waits(nc)
    return nc


_CACHE = {}


def _get_program_and_prep(x, edge_index, W, prm):
    inputs, calls, mms_by_sw = _host_prep(x, edge_index, W, prm)
    icols = sum(8 * cm.S for cm in calls)
    scols = sum(cm.S for cm in calls)
    nc = _build_program(prm, calls, mms_by_sw, icols, scols)
    return nc, inputs


def kernel(x, edge_index, W):
    prm = Prm(N=int(x.shape[0]))
    nc, inputs = _get_program_and_prep(x, edge_index, W, prm)
    res = run_bass_kernel_spmd(nc, inputs, list(range(prm.C)))
    y = np.concatenate([res.results[c]["y"] for c in range(prm.C)], axis=0)
    return y.astype(np.float32)


def run_with_trace(x, edge_index, W, trace_cores=None):
    """test.py helper: returns (y, BassKernelResults) with profiling."""
    prm = Prm(N=int(x.shape[0]))
    nc, inputs = _get_program_and_prep(x, edge_index, W, prm)
    res = run_bass_kernel_spmd(
        nc, inputs, list(range(prm.C)), trace=True, trace_cores=trace_cores
    )
    y = np.concatenate([res.results[c]["y"] for c in range(prm.C)], axis=0)
    return y.astype(np.float32), res


# revision 48
# speedup vs baseline: 1.1779x; 1.1779x over previous
"""GCN inference kernel (y = D^-1/2 A D^-1/2 (x @ W.T)) on 8 Trainium2 NeuronCores.

Strategy (full inputs in, full output out; sharded internally):
  - Destination nodes are sharded across the 8 cores (12500 dsts each);
    edges are owned by the core that owns their dst, so the segment-sum is
    core-local (per the sharding hint).
  - Phase A (sharded): each core computes the scaled projection table
    h~[n] = dinv[n] * (x[n] @ W.T) for its 12800-node shard with PE matmuls
    (bf16), writing bf16 rows padded to 256B. The table is laid out
    quarter-major so four per-quarter AllGathers pipeline with the shard
    compute and with phase B (bucket b's gathers start as soon as
    collective b lands).
  - Phase B (per core): SWDGE dma_gather streams h~[src] rows (256B each,
    only cols 0:64 carry data) for the core's dst-sorted edge list into
    SBUF; a one-hot selection matrix B (built on DVE in bf16 from dst-local
    ids vs an iota row) turns the segment-sum into PE matmuls accumulated
    in PSUM per 128-dst tile; a final per-dst dinv scale lands y.
  - One gather call per (superwindow, bucket, dst-tile) cell; the per-core
    number of REAL edges in the call is loaded into a GPSIMD register at
    runtime (num_idxs_reg) so the schedule's padding slots cost no gather
    descriptors. Pad slots keep dstl=-1 so their one-hot column is zero.
  - All data-dependent structure (edge sort, padding, gather indices,
    one-hot ids, uniform per-core slice schedule) is prepared host-side in
    numpy; the device program is identical on all 8 cores (SPMD), only the
    per-core input arrays differ.
"""

import contextlib
import math
from dataclasses import dataclass, field

import ml_dtypes
import numpy as np

import concourse.bacc as bacc
import concourse.bass as bass
import concourse.mybir as mybir
import concourse.tile as tile
from concourse import library_config
from concourse.bass_utils import run_bass_kernel_spmd

P = 128  # SBUF partitions
FIN = 128
FOUT = 64
TROW = 2 * FOUT  # table row: 64 bf16 data + 64 bf16 pad = 256B


@dataclass
class Prm:
    N: int = 100000  # nodes
    C: int = 8  # cores
    WG: int = 640  # nodes per phase-A write group
    GQ: int = 5  # write groups per quarter (pipelined collective unit)
    SWD: int = 512  # dst nodes per superwindow (TPSW * P)
    NBUFS: int = 24  # gather/one-hot pool depth (in calls)
    J: int = field(init=False)
    NS: int = field(init=False)  # dst shard size per core
    N2: int = field(init=False)  # padded node count (multiple of C*WG)
    NG: int = field(init=False)  # phase-A write groups
    NGpc: int = field(init=False)  # phase-A write groups per core
    NBK: int = field(init=False)  # gather buckets (= collective quarters)
    SHN: int = field(init=False)  # nodes per phase-A shard
    QN: int = field(init=False)  # nodes per (core, quarter)
    BKCAP: int = field(init=False)  # table rows per gather bucket
    TBLR: int = field(init=False)  # total table rows
    TPSW: int = field(init=False)  # dst tiles per superwindow
    NSW: int = field(init=False)  # superwindows per core

    def __post_init__(self):
        assert self.WG % P == 0
        assert self.SWD % P == 0
        assert self.N % self.C == 0
        self.J = self.WG // P
        self.NS = self.N // self.C
        blk = self.C * self.WG
        self.N2 = ((self.N + blk - 1) // blk) * blk
        self.NG = self.N2 // self.WG
        self.NGpc = self.NG // self.C
        assert self.NGpc % self.GQ == 0
        self.NBK = self.NGpc // self.GQ
        self.SHN = self.N2 // self.C
        self.QN = self.GQ * self.WG
        self.BKCAP = self.C * self.QN
        assert self.BKCAP <= 32767
        self.TBLR = self.N2
        self.TPSW = self.SWD // P
        self.NSW = (self.NS + self.SWD - 1) // self.SWD


def _rmap(prm, n):
    """node id -> table row, quarter-major layout: bucket k holds quarter k
    of every core's shard (so per-quarter AllGathers land contiguously)."""
    c = n // prm.SHN
    i2 = n % prm.SHN
    k = i2 // prm.QN
    i = i2 % prm.QN
    wrap = prm.WG * (i // prm.WG) + prm.J * (i % P) + (i % prm.WG) // P
    return k * prm.BKCAP + c * prm.QN + wrap


def _wrap_idx(vals16):
    """[K] int16 (K % 128 == 0) -> [128, K//16] wrapped+replicated layout."""
    k = vals16.shape[0]
    w16 = vals16.reshape(k // 16, 16).T  # [16, K/16]
    return np.tile(w16, (8, 1))  # [128, K/16]


@dataclass
class CallMeta:
    sw: int
    bk: int
    t: int
    S: int  # slices in this call (one dma_gather per call)
    icol: int  # column offset into gidx array (8 * slice offset)
    scol: int  # column offset into dstl array (slice offset)


def _schedule(prm, n_sl_u):
    """Uniform (core-independent) schedule: one gather call per non-empty
    (sw, bk, t) cell. Matmuls are emitted bucket-major per sw so PE starts
    as soon as bucket 0's gather lands; each dst-tile t accumulates into its
    own PSUM tensor across buckets (start on its first mm, stop on last).

    Returns (calls, mms_by_sw, icol_total, scol_total).
    mms_by_sw[sw] = list of (bk, t, sl, start, stop); lhsT/rhs come from
    call (sw, bk, t) local slice sl.
    """
    calls = []
    mms_by_sw = []
    icol = 0
    scol = 0
    for sw in range(prm.NSW):
        tot = [
            sum(int(n_sl_u[sw][bk][t]) for bk in range(prm.NBK))
            for t in range(prm.TPSW)
        ]
        seen = [0] * prm.TPSW
        for bk in range(prm.NBK):
            for t in range(prm.TPSW):
                S = int(n_sl_u[sw][bk][t])
                if S == 0:
                    continue
                calls.append(CallMeta(sw, bk, t, S, icol, scol))
                icol += 8 * S
                scol += S
        # matmuls grouped by dst-tile PAIR (two PSUM banks live at a time,
        # double-buffered), bucket-major within a pair
        mms = []
        for tp in range(0, prm.TPSW, 2):
            for bk in range(prm.NBK):
                for t in (tp, tp + 1):
                    if t >= prm.TPSW:
                        continue
                    for sl in range(int(n_sl_u[sw][bk][t])):
                        mms.append(
                            (bk, t, sl, seen[t] == 0, seen[t] == tot[t] - 1)
                        )
                        seen[t] += 1
        mms_by_sw.append(mms)
    return calls, mms_by_sw, icol, scol


def _host_prep(x, edge_index, W, prm):
    N, C, NS = prm.N, prm.C, prm.NS
    src = np.asarray(edge_index[0], dtype=np.int64).astype(np.int32)
    dst = np.asarray(edge_index[1], dtype=np.int64).astype(np.int32)
    x = np.asarray(x, dtype=np.float32)
    W = np.asarray(W, dtype=np.float32)

    deg = np.bincount(dst, minlength=N).astype(np.float64)
    dinv = np.where(deg > 0, 1.0 / np.sqrt(np.maximum(deg, 1.0)), 0.0).astype(
        np.float32
    )

    # gather-order node map
    r_of = _rmap(prm, np.arange(N, dtype=np.int64)).astype(np.int64)
    bk_of = (r_of // prm.BKCAP).astype(np.int32)
    rel_of = (r_of % prm.BKCAP).astype(np.int16)

    # per-edge attributes
    core_e = dst // NS
    edl = dst - core_e * NS
    sw_e = edl // prm.SWD
    t_e = (edl % prm.SWD) // P
    q_e = (edl % P).astype(np.float32)
    bk_e = bk_of[src]
    rel_e = rel_of[src]

    # per-core cell structure; edges sorted by table row within each cell
    # (HBM page locality for the gather stream)
    ncell = prm.NSW * prm.NBK * prm.TPSW
    counts = np.zeros((C, ncell), dtype=np.int64)
    percore = []
    for c in range(C):
        m = core_e == c
        order = np.lexsort((rel_e[m], t_e[m], bk_e[m], sw_e[m]))
        cell = (sw_e[m] * prm.NBK + bk_e[m]) * prm.TPSW + t_e[m]
        counts[c] = np.bincount(cell, minlength=ncell)
        percore.append(
            {
                "rel": rel_e[m][order],
                "q": q_e[m][order],
                "cell": cell[order],
            }
        )

    # uniform slice counts; ensure every in-range (sw, t) has >= 1 slice
    # somewhere so its PSUM accumulation group opens and closes
    n_sl_u = np.zeros((prm.NSW, prm.NBK, prm.TPSW), dtype=np.int64)
    cmax = counts.max(axis=0).reshape(prm.NSW, prm.NBK, prm.TPSW)
    n_sl_u[:] = (cmax + P - 1) // P
    for sw in range(prm.NSW):
        ntile = min(prm.TPSW, max(0, -(-(NS - sw * prm.SWD) // P)))
        for t in range(ntile):
            if n_sl_u[sw, :, t].sum() == 0:
                n_sl_u[sw, 0, t] = 1

    calls, mms_by_sw, icols, scols = _schedule(prm, n_sl_u)

    # slot offset (in slices) of each cell in the uniform stream
    cell_sl = n_sl_u.reshape(ncell)
    cell_off = np.zeros(ncell, dtype=np.int64)
    np.cumsum(cell_sl[:-1], out=cell_off[1:])
    S_total = int(cell_sl.sum())

    # fill per-core gather-index / dst-local / valid-count arrays
    gidx_all = np.zeros((C, P, icols), dtype=np.int16)
    dstl_all = np.full((C, P, scols), -1.0, dtype=ml_dtypes.bfloat16)
    nval_all = np.zeros((C, len(calls)), dtype=np.int32)
    cell_to_call = {}
    for ci, cm in enumerate(calls):
        cell_to_call[(cm.sw * prm.NBK + cm.bk) * prm.TPSW + cm.t] = ci
    for c in range(C):
        pc = percore[c]
        ne = pc["cell"].shape[0]
        cc = counts[c]
        starts = np.zeros(ncell, dtype=np.int64)
        np.cumsum(cc[:-1], out=starts[1:])
        rank = np.arange(ne, dtype=np.int64) - starts[pc["cell"]]
        pos = cell_off[pc["cell"]] * P + rank  # global slot position
        vals = np.full(S_total * P, -1, dtype=np.int16)
        dvals = np.full(S_total * P, -1.0, dtype=np.float32)
        vals[pos] = pc["rel"]
        dvals[pos] = pc["q"]
        for ci, cm in enumerate(calls):
            sl0 = cm.scol
            seg = vals[sl0 * P : (sl0 + cm.S) * P].copy()
            nv = int(cc[(cm.sw * prm.NBK + cm.bk) * prm.TPSW + cm.t])
            assert nv <= cm.S * P
            if nv == 0:
                # the gather ucode (and sim) need >= 1 valid index
                seg[0] = 0
                nv = 1
            nval_all[c, ci] = nv
            gidx_all[c, :, cm.icol : cm.icol + 8 * cm.S] = _wrap_idx(seg)
            dstl_all[c, :, cm.scol : cm.scol + cm.S] = (
                dvals[sl0 * P : (sl0 + cm.S) * P].reshape(cm.S, P).T
            )

    # phase-A inputs
    xT = np.zeros((FIN, prm.N2), dtype=ml_dtypes.bfloat16)
    xT[:, :N] = x.T.astype(ml_dtypes.bfloat16)
    WT = np.ascontiguousarray(W.T).astype(ml_dtypes.bfloat16)  # [FIN, FOUT]
    dinvA = np.zeros((P, prm.NG * prm.J), dtype=np.float32)
    n_idx = np.arange(prm.N2)
    g_i, j_i, p_i = n_idx // prm.WG, (n_idx % prm.WG) // P, n_idx % P
    dpad = np.zeros(prm.N2, dtype=np.float32)
    dpad[:N] = dinv
    dinvA[p_i, g_i * prm.J + j_i] = dpad
    iota = np.broadcast_to(
        np.arange(P, dtype=ml_dtypes.bfloat16)[None, :], (P, P)
    ).copy()
    dinvD = np.zeros((C, P, prm.NSW * prm.TPSW), dtype=np.float32)
    w_idx = np.arange(prm.NSW * prm.TPSW)
    for c in range(C):
        node = c * NS + w_idx[:, None] * P + np.arange(P)[None, :]
        ok = node < (c + 1) * NS
        dv = np.where(ok, dinv[np.minimum(node, N - 1)], 0.0)
        dinvD[c][np.arange(P)[None, :], w_idx[:, None]] = dv

    inputs = []
    for c in range(C):
        inputs.append(
            {
                "xT": xT,  # phase A replicated: full table on every core
                "WT": WT,
                "dinvA": dinvA,
                "iota": iota,
                "dinvD": dinvD[c],
                "gidx": gidx_all[c],
                "dstl": dstl_all[c],
                "nval": nval_all[c : c + 1],  # [1, ncalls]
            }
        )
    return inputs, calls, mms_by_sw


def _split_sync_waits(nc):
    """This env's walrus rejects >1 sync wait on some opcodes; keep 1 wait
    per instruction, moving extras onto preceding same-engine NOPs."""
    for bb in nc.main_func.blocks:
        insts = bb.instructions
        i = 0
        while i < len(insts):
            ins = insts[i]
            si = ins.sync_info
            if si is not None and si.on_wait is not None and len(si.on_wait) > 1:
                waits = list(si.on_wait)
                keep, extra = waits[-1:], waits[:-1]
                k = 0
                while extra:
                    chunk, extra = extra[:1], extra[1:]
                    nop = mybir.InstNoOp(name=f"{ins.name}-ws{k}", ins=[], outs=[])
                    nop.engine = ins.engine
                    nop.sync_info = mybir.SyncInfo(on_wait=chunk, on_update=[])
                    nc.register_instruction(nop)
                    insts.insert(i, nop)
                    i += 1
                    k += 1
                ins.sync_info = mybir.SyncInfo(
                    on_wait=keep, on_update=list(si.on_update or [])
                )
            i += 1


def _build_program(prm, calls, mms_by_sw, icols, scols, mode="full"):
    f32 = mybir.dt.float32
    bf16 = mybir.dt.bfloat16
    i32 = mybir.dt.int32
    nc = bacc.Bacc("TRN2", num_swdge_queues=4)

    ncalls = max(1, len(calls))
    xT = nc.declare_dram_parameter("xT", [FIN, prm.N2], bf16, isOutput=False)
    WT = nc.declare_dram_parameter("WT", [FIN, FOUT], bf16, isOutput=False)
    dinvA = nc.declare_dram_parameter(
        "dinvA", [P, prm.NG * prm.J], f32, isOutput=False
    )
    iota = nc.declare_dram_parameter("iota", [P, P], bf16, isOutput=False)
    dinvD = nc.declare_dram_parameter(
        "dinvD", [P, prm.NSW * prm.TPSW], f32, isOutput=False
    )
    gidx = nc.declare_dram_parameter("gidx", [P, icols], mybir.dt.int16, isOutput=False)
    dstl = nc.declare_dram_parameter("dstl", [P, scols], bf16, isOutput=False)
    nval = nc.declare_dram_parameter("nval", [1, ncalls], i32, isOutput=False)
    y = nc.declare_dram_parameter("y", [prm.NS, FOUT], f32, isOutput=True)
    with tile.TileContext(nc) as tc:
        # phase A is REPLICATED: every core computes the full table itself
        # (26MB xT stream overlaps early gathers; no collective rendezvous).
        # Tile-tracked DRAM tiles so bucket k's gathers automatically wait
        # for its 40 table-write DMAs (Tile doesn't track raw dram_tensors).
        TBL = [
            tc.tile(
                [prm.BKCAP, TROW], bf16, space="DRAM", name=f"tbl{k}", tag=f"tbl{k}"
            )[0]
            for k in range(prm.NBK)
        ]
        with tc.tile_pool(name="const", bufs=1) as cpool:
            wt_sb = cpool.tile([FIN, FOUT], bf16, tag="wt")
            nc.sync.dma_start(out=wt_sb[:], in_=WT[:])
            dinvA_sb = cpool.tile([P, prm.NG * prm.J], f32, tag="da")
            nc.sync.dma_start(out=dinvA_sb[:], in_=dinvA[:])
            iota_sb = cpool.tile([P, P], bf16, tag="io")
            nc.sync.dma_start(out=iota_sb[:], in_=iota[:])
            dinvD_sb = cpool.tile([P, prm.NSW * prm.TPSW], f32, tag="dd")
            nc.sync.dma_start(out=dinvD_sb[:], in_=dinvD[:])
            nval_sb = cpool.tile([1, ncalls], i32, tag="nv")
            nc.sync.dma_start(out=nval_sb[:], in_=nval[:])

            # All pools are opened together: closing phase A's pools before
            # opening phase B's would recycle their SBUF into the new pools,
            # and Tile then serializes all of phase B behind the last
            # phase-A reader of that space.
            _stk = contextlib.ExitStack()
            pa = _stk.enter_context(tc.tile_pool(name="pa", bufs=4))
            psa = _stk.enter_context(tc.tile_pool(name="psa", bufs=2, space="PSUM"))
            pidx = _stk.enter_context(tc.tile_pool(name="pidx", bufs=prm.NBUFS))
            pg = _stk.enter_context(tc.tile_pool(name="pg", bufs=prm.NBUFS))
            pb = _stk.enter_context(tc.tile_pool(name="pb", bufs=prm.NBUFS))
            py = _stk.enter_context(tc.tile_pool(name="py", bufs=3))
            psb = _stk.enter_context(tc.tile_pool(name="psb", bufs=2, space="PSUM"))

            # ------- Phase A: full table per core, bucket-major -----------
            ng_emitted = 0
            for k in range(prm.NBK):
                for c2 in range(prm.C):
                    for gq in range(prm.GQ):
                        # nodes c2*SHN + k*QN + gq*WG + [0, WG)
                        n0 = c2 * prm.SHN + k * prm.QN + gq * prm.WG
                        g = n0 // prm.WG
                        xt = pa.tile([P, prm.WG], bf16, tag="xt")
                        nc.sync.dma_start(
                            out=xt[:], in_=xT[:, n0 : n0 + prm.WG]
                        )
                        hps = psa.tile([P, prm.J * FOUT], f32, tag="hps")
                        for j in range(prm.J):
                            nc.tensor.matmul(
                                out=hps[:, j * FOUT : (j + 1) * FOUT],
                                lhsT=xt[:, j * P : (j + 1) * P],
                                rhs=wt_sb[:],
                                start=True,
                                stop=True,
                            )
                        tsb = pa.tile([P, prm.J, TROW], bf16, tag="tsb")
                        if ng_emitted < 4:  # zero pad cols once per buffer
                            nc.vector.memset(tsb[:], 0.0)
                        ng_emitted += 1
                        nc.vector.tensor_tensor(
                            out=tsb[:, :, :FOUT],
                            in0=hps[:].rearrange("p (j f) -> p j f", f=FOUT),
                            in1=dinvA_sb[:, g * prm.J : (g + 1) * prm.J][
                                :, :, None
                            ].to_broadcast([P, prm.J, FOUT]),
                            op=mybir.AluOpType.mult,
                        )
                        base = c2 * prm.QN + gq * prm.WG
                        nc.sync.dma_start(
                            out=TBL[k][base : base + prm.WG, :].rearrange(
                                "(p j) f -> p j f", j=prm.J
                            ),
                            in_=tsb[:],
                        )

            # ---------------- Phase B: gather + segment-sum ----------------
            if mode == "phaseA":
                with tc.tile_pool(name="pz", bufs=1) as pz:
                    zy = pz.tile([P, FOUT], f32, tag="zy")
                    nc.vector.memset(zy[:], 0.0)
                    for r0 in range(0, prm.NS, P):
                        rt = min(P, prm.NS - r0)
                        nc.sync.dma_start(out=y[r0 : r0 + rt, :], in_=zy[:rt, :])
                calls = []
            S_MAX = max((cm.S for cm in calls), default=1)
            calls_by_swbk = {}
            for ci, cm in enumerate(calls):
                calls_by_swbk.setdefault((cm.sw, cm.bk), []).append((ci, cm))
            nreg = nc.gpsimd.alloc_register("nvreg")
            qctr = [0]
            emitted = [0]
            bk_waited = set()

            def emit_calls(sw, bk, tiles):
                for ci, cm in calls_by_swbk.get((sw, bk), []):
                    S = cm.S
                    idx_t = pidx.tile([P, 8 * S_MAX], mybir.dt.int16, tag="idx")
                    nc.scalar.dma_start(
                        out=idx_t[:, : 8 * S],
                        in_=gidx[:, cm.icol : cm.icol + 8 * S],
                    )
                    dst_t = pidx.tile([P, S_MAX], bf16, tag="dst")
                    nc.scalar.dma_start(
                        out=dst_t[:, :S], in_=dstl[:, cm.scol : cm.scol + S]
                    )
                    g_t = pg.tile([P, S_MAX, TROW], bf16, tag="g")
                    # zero each pool buffer on its first use: slots skipped by
                    # the runtime-count gather must hold finite data (0*NaN
                    # would poison the PE accumulation)
                    if emitted[0] < prm.NBUFS:
                        nc.vector.memset(g_t[:], 0.0)
                    emitted[0] += 1
                    if mode == "nogather":
                        nc.vector.memset(g_t[:, :S, :], 0.0)
                    else:
                        nc.gpsimd.reg_load(nreg, nval_sb[0:1, ci : ci + 1])
                        nc.gpsimd.dma_gather(
                            out_ap=g_t[:, :S, :],
                            in_ap=TBL[cm.bk][:],
                            idxs_ap=idx_t[:, : 8 * S],
                            num_idxs=S * P,
                            num_idxs_reg=nreg,
                            elem_size=TROW,
                            single_packet=False,
                            queue_num=qctr[0] % 4,
                        )
                        qctr[0] += 1
                    b_t = pb.tile([P, S_MAX, P], bf16, tag="b")
                    nc.vector.tensor_tensor(
                        out=b_t[:, :S, :],
                        in0=dst_t[:, :S][:, :, None].to_broadcast([P, S, P]),
                        in1=iota_sb[:, None, :].to_broadcast([P, S, P]),
                        op=mybir.AluOpType.is_equal,
                    )
                    tiles[(cm.bk, cm.t)] = (g_t, b_t)

            def emit_mms(sw, tiles):
                # accumulation groups must never share a PSUM bank
                # (start=True clears the whole bank) - one [P, FOUT] tile per
                # open dst tile, two open at a time (tag acc0/acc1, bufs=2)
                rows_sw = min(prm.SWD, prm.NS - sw * prm.SWD)
                nt = (rows_sw + P - 1) // P  # valid dst tiles this sw
                ysb = py.tile([P, prm.TPSW, FOUT], f32, tag="ysb")
                mms = mms_by_sw[sw]
                for tp in range(0, prm.TPSW, 2):
                    pair = [t for t in (tp, tp + 1) if t < prm.TPSW]
                    psum_t = {
                        t: psb.tile(
                            [P, FOUT], f32, tag=f"acc{t % 2}", name=f"acc{t}"
                        )
                        for t in pair
                    }
                    for bk, t, sl, st, sp in mms:
                        if t not in pair:
                            continue
                        g_t, b_t = tiles[(bk, t)]
                        nc.tensor.matmul(
                            out=psum_t[t][:],
                            lhsT=b_t[:, sl, :],
                            rhs=g_t[:, sl, :FOUT],
                            start=st,
                            stop=sp,
                        )
                    # scale by dinv[dst] on the otherwise-idle Scalar engine
                    for t in pair:
                        if t >= nt:
                            continue
                        w = sw * prm.TPSW + t
                        nc.scalar.activation(
                            out=ysb[:, t, :],
                            in_=psum_t[t][:],
                            func=mybir.ActivationFunctionType.Copy,
                            scale=dinvD_sb[:, w : w + 1],
                        )
                for t in range(nt):
                    rt = min(P, rows_sw - t * P)
                    r0 = sw * prm.SWD + t * P
                    nc.sync.dma_start(out=y[r0 : r0 + rt, :], in_=ysb[:rt, t, :])

            for sw in range(prm.NSW):
                tiles = {}
                for bk in range(prm.NBK):
                    emit_calls(sw, bk, tiles)
                emit_mms(sw, tiles)
            _stk.close()

    nc.compile()
    _split_sync_waits(nc)
    return nc


_CACHE = {}


def _get_program_and_prep(x, edge_index, W, prm):
    inputs, calls, mms_by_sw = _host_prep(x, edge_index, W, prm)
    icols = sum(8 * cm.S for cm in calls)
    scols = sum(cm.S for cm in calls)
    nc = _build_program(prm, calls, mms_by_sw, icols, scols)
    return nc, inputs


def kernel(x, edge_index, W):
    prm = Prm(N=int(x.shape[0]))
    nc, inputs = _get_program_and_prep(x, edge_index, W, prm)
    res = run_bass_kernel_spmd(nc, inputs, list(range(prm.C)))
    y = np.concatenate([res.results[c]["y"] for c in range(prm.C)], axis=0)
    return y.astype(np.float32)


def run_with_trace(x, edge_index, W, trace_cores=None):
    """test.py helper: returns (y, BassKernelResults) with profiling."""
    prm = Prm(N=int(x.shape[0]))
    nc, inputs = _get_program_and_prep(x, edge_index, W, prm)
    res = run_bass_kernel_spmd(
        nc, inputs, list(range(prm.C)), trace=True, trace_cores=trace_cores
    )
    y = np.concatenate([res.results[c]["y"] for c in range(prm.C)], axis=0)
    return y.astype(np.float32), res


# revision 57
# speedup vs baseline: 1.2797x; 1.0865x over previous
"""GCN inference kernel (y = D^-1/2 A D^-1/2 (x @ W.T)) on 8 Trainium2 NeuronCores.

Strategy (full inputs in, full output out; sharded internally):
  - Destination nodes are sharded across the 8 cores (12500 dsts each);
    edges are owned by the core that owns their dst, so the segment-sum is
    core-local (per the sharding hint).
  - Phase A (sharded): each core computes the scaled projection table
    h~[n] = dinv[n] * (x[n] @ W.T) for its 12800-node shard with PE matmuls
    (bf16), writing bf16 rows padded to 256B. The table is laid out
    quarter-major so four per-quarter AllGathers pipeline with the shard
    compute and with phase B (bucket b's gathers start as soon as
    collective b lands).
  - Phase B (per core): SWDGE dma_gather streams h~[src] rows (256B each,
    only cols 0:64 carry data) for the core's dst-sorted edge list into
    SBUF; a one-hot selection matrix B (built on DVE in bf16 from dst-local
    ids vs an iota row) turns the segment-sum into PE matmuls accumulated
    in PSUM per 128-dst tile; a final per-dst dinv scale lands y.
  - One gather call per (superwindow, bucket, dst-tile) cell; the per-core
    number of REAL edges in the call is loaded into a GPSIMD register at
    runtime (num_idxs_reg) so the schedule's padding slots cost no gather
    descriptors. Pad slots keep dstl=-1 so their one-hot column is zero.
  - All data-dependent structure (edge sort, padding, gather indices,
    one-hot ids, uniform per-core slice schedule) is prepared host-side in
    numpy; the device program is identical on all 8 cores (SPMD), only the
    per-core input arrays differ.
"""

import contextlib
import math
from dataclasses import dataclass, field

import ml_dtypes
import numpy as np

import concourse.bacc as bacc
import concourse.bass as bass
import concourse.mybir as mybir
import concourse.tile as tile
from concourse import library_config
from concourse.bass_utils import run_bass_kernel_spmd

P = 128  # SBUF partitions
FIN = 128
FOUT = 64
TROW = 2 * FOUT  # table row: 64 bf16 data + 64 bf16 pad = 256B


@dataclass
class Prm:
    N: int = 100000  # nodes
    C: int = 8  # cores
    WG: int = 640  # nodes per phase-A write group
    GQ: int = 5  # write groups per quarter (pipelined collective unit)
    SWD: int = 512  # dst nodes per superwindow (TPSW * P)
    NBUFS: int = 30  # gather/one-hot pool depth (in calls)
    J: int = field(init=False)
    NS: int = field(init=False)  # dst shard size per core
    N2: int = field(init=False)  # padded node count (multiple of C*WG)
    NG: int = field(init=False)  # phase-A write groups
    NGpc: int = field(init=False)  # phase-A write groups per core
    NBK: int = field(init=False)  # gather buckets (= collective quarters)
    SHN: int = field(init=False)  # nodes per phase-A shard
    QN: int = field(init=False)  # nodes per (core, quarter)
    BKCAP: int = field(init=False)  # table rows per gather bucket
    TBLR: int = field(init=False)  # total table rows
    TPSW: int = field(init=False)  # dst tiles per superwindow
    NSW: int = field(init=False)  # superwindows per core

    def __post_init__(self):
        assert self.WG % P == 0
        assert self.SWD % P == 0
        assert self.N % self.C == 0
        self.J = self.WG // P
        self.NS = self.N // self.C
        blk = self.C * self.WG
        self.N2 = ((self.N + blk - 1) // blk) * blk
        self.NG = self.N2 // self.WG
        self.NGpc = self.NG // self.C
        assert self.NGpc % self.GQ == 0
        self.NBK = self.NGpc // self.GQ
        self.SHN = self.N2 // self.C
        self.QN = self.GQ * self.WG
        self.BKCAP = self.C * self.QN
        assert self.BKCAP <= 32767
        self.TBLR = self.N2
        self.TPSW = self.SWD // P
        self.NSW = (self.NS + self.SWD - 1) // self.SWD


def _rmap(prm, n):
    """node id -> table row, quarter-major layout: bucket k holds quarter k
    of every core's shard (so per-quarter AllGathers land contiguously)."""
    c = n // prm.SHN
    i2 = n % prm.SHN
    k = i2 // prm.QN
    i = i2 % prm.QN
    wrap = prm.WG * (i // prm.WG) + prm.J * (i % P) + (i % prm.WG) // P
    return k * prm.BKCAP + c * prm.QN + wrap


def _wrap_idx(vals16):
    """[K] int16 (K % 128 == 0) -> [128, K//16] wrapped+replicated layout."""
    k = vals16.shape[0]
    w16 = vals16.reshape(k // 16, 16).T  # [16, K/16]
    return np.tile(w16, (8, 1))  # [128, K/16]


@dataclass
class CallMeta:
    sw: int
    bk: int
    t: int
    S: int  # slices in this call (one dma_gather per call)
    icol: int  # column offset into gidx array (8 * slice offset)
    scol: int  # column offset into dstl array (slice offset)


def _schedule(prm, n_sl_u):
    """Uniform (core-independent) schedule: one gather call per non-empty
    (sw, bk, t) cell. Matmuls are emitted bucket-major per sw so PE starts
    as soon as bucket 0's gather lands; each dst-tile t accumulates into its
    own PSUM tensor across buckets (start on its first mm, stop on last).

    Returns (calls, mms_by_sw, icol_total, scol_total).
    mms_by_sw[sw] = list of (bk, t, sl, start, stop); lhsT/rhs come from
    call (sw, bk, t) local slice sl.
    """
    calls = []
    mms_by_sw = []
    icol = 0
    scol = 0
    for sw in range(prm.NSW):
        tot = [
            sum(int(n_sl_u[sw][bk][t]) for bk in range(prm.NBK))
            for t in range(prm.TPSW)
        ]
        seen = [0] * prm.TPSW
        for bk in range(prm.NBK):
            for t in range(prm.TPSW):
                S = int(n_sl_u[sw][bk][t])
                if S == 0:
                    continue
                calls.append(CallMeta(sw, bk, t, S, icol, scol))
                icol += 8 * S
                scol += S
        # matmuls grouped by dst-tile PAIR (two PSUM banks live at a time,
        # double-buffered), bucket-major within a pair
        mms = []
        for tp in range(0, prm.TPSW, 2):
            for bk in range(prm.NBK):
                for t in (tp, tp + 1):
                    if t >= prm.TPSW:
                        continue
                    for sl in range(int(n_sl_u[sw][bk][t])):
                        mms.append(
                            (bk, t, sl, seen[t] == 0, seen[t] == tot[t] - 1)
                        )
                        seen[t] += 1
        mms_by_sw.append(mms)
    return calls, mms_by_sw, icol, scol


def _host_prep(x, edge_index, W, prm):
    N, C, NS = prm.N, prm.C, prm.NS
    src = np.asarray(edge_index[0], dtype=np.int64).astype(np.int32)
    dst = np.asarray(edge_index[1], dtype=np.int64).astype(np.int32)
    x = np.asarray(x, dtype=np.float32)
    W = np.asarray(W, dtype=np.float32)

    deg = np.bincount(dst, minlength=N).astype(np.float64)
    dinv = np.where(deg > 0, 1.0 / np.sqrt(np.maximum(deg, 1.0)), 0.0).astype(
        np.float32
    )

    # gather-order node map
    r_of = _rmap(prm, np.arange(N, dtype=np.int64)).astype(np.int64)
    bk_of = (r_of // prm.BKCAP).astype(np.int32)
    rel_of = (r_of % prm.BKCAP).astype(np.int16)

    # per-edge attributes
    core_e = dst // NS
    edl = dst - core_e * NS
    sw_e = edl // prm.SWD
    t_e = (edl % prm.SWD) // P
    q_e = (edl % P).astype(np.float32)
    bk_e = bk_of[src]
    rel_e = rel_of[src]

    # per-core cell structure; edges sorted by table row within each cell
    # (HBM page locality for the gather stream)
    ncell = prm.NSW * prm.NBK * prm.TPSW
    counts = np.zeros((C, ncell), dtype=np.int64)
    percore = []
    for c in range(C):
        m = core_e == c
        order = np.lexsort((rel_e[m], t_e[m], bk_e[m], sw_e[m]))
        cell = (sw_e[m] * prm.NBK + bk_e[m]) * prm.TPSW + t_e[m]
        counts[c] = np.bincount(cell, minlength=ncell)
        percore.append(
            {
                "rel": rel_e[m][order],
                "q": q_e[m][order],
                "cell": cell[order],
            }
        )

    # uniform slice counts; ensure every in-range (sw, t) has >= 1 slice
    # somewhere so its PSUM accumulation group opens and closes
    n_sl_u = np.zeros((prm.NSW, prm.NBK, prm.TPSW), dtype=np.int64)
    cmax = counts.max(axis=0).reshape(prm.NSW, prm.NBK, prm.TPSW)
    n_sl_u[:] = (cmax + P - 1) // P
    for sw in range(prm.NSW):
        ntile = min(prm.TPSW, max(0, -(-(NS - sw * prm.SWD) // P)))
        for t in range(ntile):
            if n_sl_u[sw, :, t].sum() == 0:
                n_sl_u[sw, 0, t] = 1

    calls, mms_by_sw, icols, scols = _schedule(prm, n_sl_u)

    # slot offset (in slices) of each cell in the uniform stream
    cell_sl = n_sl_u.reshape(ncell)
    cell_off = np.zeros(ncell, dtype=np.int64)
    np.cumsum(cell_sl[:-1], out=cell_off[1:])
    S_total = int(cell_sl.sum())

    # fill per-core gather-index / dst-local / valid-count arrays
    gidx_all = np.zeros((C, P, icols), dtype=np.int16)
    dstl_all = np.full((C, P, scols), -1.0, dtype=ml_dtypes.bfloat16)
    nval_all = np.zeros((C, len(calls)), dtype=np.int32)
    cell_to_call = {}
    for ci, cm in enumerate(calls):
        cell_to_call[(cm.sw * prm.NBK + cm.bk) * prm.TPSW + cm.t] = ci
    for c in range(C):
        pc = percore[c]
        ne = pc["cell"].shape[0]
        cc = counts[c]
        starts = np.zeros(ncell, dtype=np.int64)
        np.cumsum(cc[:-1], out=starts[1:])
        rank = np.arange(ne, dtype=np.int64) - starts[pc["cell"]]
        pos = cell_off[pc["cell"]] * P + rank  # global slot position
        vals = np.full(S_total * P, -1, dtype=np.int16)
        dvals = np.full(S_total * P, -1.0, dtype=np.float32)
        vals[pos] = pc["rel"]
        dvals[pos] = pc["q"]
        for ci, cm in enumerate(calls):
            sl0 = cm.scol
            seg = vals[sl0 * P : (sl0 + cm.S) * P].copy()
            nv = int(cc[(cm.sw * prm.NBK + cm.bk) * prm.TPSW + cm.t])
            assert nv <= cm.S * P
            if nv == 0:
                # the gather ucode (and sim) need >= 1 valid index
                seg[0] = 0
                nv = 1
            nval_all[c, ci] = nv
            gidx_all[c, :, cm.icol : cm.icol + 8 * cm.S] = _wrap_idx(seg)
            dstl_all[c, :, cm.scol : cm.scol + cm.S] = (
                dvals[sl0 * P : (sl0 + cm.S) * P].reshape(cm.S, P).T
            )

    # phase-A inputs
    xT = np.zeros((FIN, prm.N2), dtype=ml_dtypes.bfloat16)
    xT[:, :N] = x.T.astype(ml_dtypes.bfloat16)
    WT = np.ascontiguousarray(W.T).astype(ml_dtypes.bfloat16)  # [FIN, FOUT]
    dpad = np.zeros(prm.N2, dtype=np.float32)
    dpad[:N] = dinv
    iota = np.broadcast_to(
        np.arange(P, dtype=ml_dtypes.bfloat16)[None, :], (P, P)
    ).copy()
    dinvD = np.zeros((C, P, prm.NSW * prm.TPSW), dtype=np.float32)
    w_idx = np.arange(prm.NSW * prm.TPSW)
    for c in range(C):
        node = c * NS + w_idx[:, None] * P + np.arange(P)[None, :]
        ok = node < (c + 1) * NS
        dv = np.where(ok, dinv[np.minimum(node, N - 1)], 0.0)
        dinvD[c][np.arange(P)[None, :], w_idx[:, None]] = dv

    # phase-A shard for core c: the nodes whose table rows fall in its
    # AllGather output block [SHN*c, SHN*(c+1)) of the quarter-major layout:
    # quarter (c//2) of original node shards 4*(c%2) .. 4*(c%2)+3
    inputs = []
    i2 = np.arange(prm.SHN)
    for c in range(C):
        segs = [
            np.arange(prm.QN, dtype=np.int64)
            + (4 * (c % 2) + u) * prm.SHN
            + (c // 2) * prm.QN
            for u in range(C // 2)
        ]
        nodes = np.concatenate(segs)  # SHN nodes in TBSH write order
        assert nodes.shape[0] == prm.SHN
        seq = dpad[nodes]
        dinvA_c = np.zeros((P, prm.NGpc * prm.J), dtype=np.float32)
        dinvA_c[i2 % P, (i2 // prm.WG) * prm.J + (i2 % prm.WG) // P] = seq
        inputs.append(
            {
                "xT": np.ascontiguousarray(xT[:, nodes]),
                "WT": WT,
                "dinvA": dinvA_c,
                "iota": iota,
                "dinvD": dinvD[c],
                "gidx": gidx_all[c],
                "dstl": dstl_all[c],
                "nval": nval_all[c : c + 1],  # [1, ncalls]
            }
        )
    return inputs, calls, mms_by_sw


def _split_sync_waits(nc):
    """This env's walrus rejects >1 sync wait on some opcodes; keep 1 wait
    per instruction, moving extras onto preceding same-engine NOPs."""
    for bb in nc.main_func.blocks:
        insts = bb.instructions
        i = 0
        while i < len(insts):
            ins = insts[i]
            si = ins.sync_info
            if si is not None and si.on_wait is not None and len(si.on_wait) > 1:
                waits = list(si.on_wait)
                keep, extra = waits[-1:], waits[:-1]
                k = 0
                while extra:
                    chunk, extra = extra[:1], extra[1:]
                    nop = mybir.InstNoOp(name=f"{ins.name}-ws{k}", ins=[], outs=[])
                    nop.engine = ins.engine
                    nop.sync_info = mybir.SyncInfo(on_wait=chunk, on_update=[])
                    nc.register_instruction(nop)
                    insts.insert(i, nop)
                    i += 1
                    k += 1
                ins.sync_info = mybir.SyncInfo(
                    on_wait=keep, on_update=list(si.on_update or [])
                )
            i += 1


def _build_program(prm, calls, mms_by_sw, icols, scols, mode="full"):
    f32 = mybir.dt.float32
    bf16 = mybir.dt.bfloat16
    i32 = mybir.dt.int32
    nc = bacc.Bacc("TRN2", num_swdge_queues=4)

    NGpc = prm.NGpc
    ncalls = max(1, len(calls))
    xT = nc.declare_dram_parameter(
        "xT", [FIN, NGpc * prm.WG], bf16, isOutput=False
    )
    WT = nc.declare_dram_parameter("WT", [FIN, FOUT], bf16, isOutput=False)
    dinvA = nc.declare_dram_parameter(
        "dinvA", [P, NGpc * prm.J], f32, isOutput=False
    )
    iota = nc.declare_dram_parameter("iota", [P, P], bf16, isOutput=False)
    dinvD = nc.declare_dram_parameter(
        "dinvD", [P, prm.NSW * prm.TPSW], f32, isOutput=False
    )
    gidx = nc.declare_dram_parameter("gidx", [P, icols], mybir.dt.int16, isOutput=False)
    dstl = nc.declare_dram_parameter("dstl", [P, scols], bf16, isOutput=False)
    nval = nc.declare_dram_parameter("nval", [1, ncalls], i32, isOutput=False)
    y = nc.declare_dram_parameter("y", [prm.NS, FOUT], f32, isOutput=True)
    # phase A is SHARDED: each core computes its 12800-row table shard, an
    # AllGather assembles the full table. Phase A + collective finish in
    # ~110us; overlapping phase A with the gather stream measured SLOWER
    # (HWDGE packets steal SDMA dispatch slots from the SWDGE gathers), so
    # a short serial phase A beats a long overlapped one.
    TBSH = nc.dram_tensor("tbsh", [NGpc * prm.WG, TROW], bf16)
    TBLA = nc.dram_tensor(
        "tbla", [prm.TBLR, TROW], bf16, addr_space="Shared"
    )

    with tile.TileContext(nc) as tc:
        with tc.tile_pool(name="const", bufs=1) as cpool:
            wt_sb = cpool.tile([FIN, FOUT], bf16, tag="wt")
            nc.sync.dma_start(out=wt_sb[:], in_=WT[:])
            dinvA_sb = cpool.tile([P, NGpc * prm.J], f32, tag="da")
            nc.sync.dma_start(out=dinvA_sb[:], in_=dinvA[:])
            iota_sb = cpool.tile([P, P], bf16, tag="io")
            nc.sync.dma_start(out=iota_sb[:], in_=iota[:])
            dinvD_sb = cpool.tile([P, prm.NSW * prm.TPSW], f32, tag="dd")
            nc.sync.dma_start(out=dinvD_sb[:], in_=dinvD[:])
            nval_sb = cpool.tile([1, ncalls], i32, tag="nv")
            nc.sync.dma_start(out=nval_sb[:], in_=nval[:])

            # ------- Phase A: build the table shard, then AllGather -------
            with (
                tc.tile_pool(name="pa", bufs=4) as pa,
                tc.tile_pool(name="psa", bufs=2, space="PSUM") as psa,
            ):
                for g in range(NGpc):
                    xt = pa.tile([P, prm.WG], bf16, tag="xt")
                    nc.sync.dma_start(
                        out=xt[:], in_=xT[:, g * prm.WG : (g + 1) * prm.WG]
                    )
                    hps = psa.tile([P, prm.J * FOUT], f32, tag="hps")
                    for j in range(prm.J):
                        nc.tensor.matmul(
                            out=hps[:, j * FOUT : (j + 1) * FOUT],
                            lhsT=xt[:, j * P : (j + 1) * P],
                            rhs=wt_sb[:],
                            start=True,
                            stop=True,
                        )
                    tsb = pa.tile([P, prm.J, TROW], bf16, tag="tsb")
                    if g < 4:  # zero pad cols once per buffer
                        nc.vector.memset(tsb[:], 0.0)
                    nc.vector.tensor_tensor(
                        out=tsb[:, :, :FOUT],
                        in0=hps[:].rearrange("p (j f) -> p j f", f=FOUT),
                        in1=dinvA_sb[:, g * prm.J : (g + 1) * prm.J][
                            :, :, None
                        ].to_broadcast([P, prm.J, FOUT]),
                        op=mybir.AluOpType.mult,
                    )
                    base = prm.WG * g
                    nc.sync.dma_start(
                        out=TBSH[base : base + prm.WG, :].rearrange(
                            "(p j) f -> p j f", j=prm.J
                        ),
                        in_=tsb[:],
                    )

            # assemble the full table from all cores' shards
            nc.gpsimd.collective_compute(
                "AllGather",
                mybir.AluOpType.bypass,
                replica_groups=[list(range(prm.C))],
                ins=[TBSH[:]],
                outs=[TBLA[:]],
            )
            TBL = [
                TBLA[k * prm.BKCAP : (k + 1) * prm.BKCAP, :]
                for k in range(prm.NBK)
            ]

            _stk = contextlib.ExitStack()
            pidx = _stk.enter_context(tc.tile_pool(name="pidx", bufs=prm.NBUFS))
            pg = _stk.enter_context(tc.tile_pool(name="pg", bufs=prm.NBUFS))
            pb = _stk.enter_context(tc.tile_pool(name="pb", bufs=prm.NBUFS))
            py = _stk.enter_context(tc.tile_pool(name="py", bufs=3))
            psb = _stk.enter_context(tc.tile_pool(name="psb", bufs=2, space="PSUM"))

            # ---------------- Phase B: gather + segment-sum ----------------
            if mode == "phaseA":
                with tc.tile_pool(name="pz", bufs=1) as pz:
                    zy = pz.tile([P, FOUT], f32, tag="zy")
                    nc.vector.memset(zy[:], 0.0)
                    for r0 in range(0, prm.NS, P):
                        rt = min(P, prm.NS - r0)
                        nc.sync.dma_start(out=y[r0 : r0 + rt, :], in_=zy[:rt, :])
                calls = []
            S_MAX = max((cm.S for cm in calls), default=1)
            calls_by_swbk = {}
            for ci, cm in enumerate(calls):
                calls_by_swbk.setdefault((cm.sw, cm.bk), []).append((ci, cm))
            nreg = nc.gpsimd.alloc_register("nvreg")
            qctr = [0]
            emitted = [0]
            bk_waited = set()

            def emit_calls(sw, bk, tiles):
                for ci, cm in calls_by_swbk.get((sw, bk), []):
                    S = cm.S
                    idx_t = pidx.tile([P, 8 * S_MAX], mybir.dt.int16, tag="idx")
                    nc.scalar.dma_start(
                        out=idx_t[:, : 8 * S],
                        in_=gidx[:, cm.icol : cm.icol + 8 * S],
                    )
                    dst_t = pidx.tile([P, S_MAX], bf16, tag="dst")
                    nc.scalar.dma_start(
                        out=dst_t[:, :S], in_=dstl[:, cm.scol : cm.scol + S]
                    )
                    g_t = pg.tile([P, S_MAX, TROW], bf16, tag="g")
                    # zero each pool buffer on its first use: slots skipped by
                    # the runtime-count gather must hold finite data (0*NaN
                    # would poison the PE accumulation)
                    if emitted[0] < prm.NBUFS:
                        nc.vector.memset(g_t[:], 0.0)
                    emitted[0] += 1
                    if mode == "nogather":
                        nc.vector.memset(g_t[:, :S, :], 0.0)
                    else:
                        nc.gpsimd.reg_load(nreg, nval_sb[0:1, ci : ci + 1])
                        nc.gpsimd.dma_gather(
                            out_ap=g_t[:, :S, :],
                            in_ap=TBL[cm.bk],
                            idxs_ap=idx_t[:, : 8 * S],
                            num_idxs=S * P,
                            num_idxs_reg=nreg,
                            elem_size=TROW,
                            single_packet=False,
                            queue_num=qctr[0] % 4,
                        )
                        qctr[0] += 1
                    b_t = pb.tile([P, S_MAX, P], bf16, tag="b")
                    nc.vector.tensor_tensor(
                        out=b_t[:, :S, :],
                        in0=dst_t[:, :S][:, :, None].to_broadcast([P, S, P]),
                        in1=iota_sb[:, None, :].to_broadcast([P, S, P]),
                        op=mybir.AluOpType.is_equal,
                    )
                    tiles[(cm.bk, cm.t)] = (g_t, b_t)

            def emit_mms(sw, tiles):
                # accumulation groups must never share a PSUM bank
                # (start=True clears the whole bank) - one [P, FOUT] tile per
                # open dst tile, two open at a time (tag acc0/acc1, bufs=2)
                rows_sw = min(prm.SWD, prm.NS - sw * prm.SWD)
                nt = (rows_sw + P - 1) // P  # valid dst tiles this sw
                ysb = py.tile([P, prm.TPSW, FOUT], f32, tag="ysb")
                mms = mms_by_sw[sw]
                for tp in range(0, prm.TPSW, 2):
                    pair = [t for t in (tp, tp + 1) if t < prm.TPSW]
                    psum_t = {
                        t: psb.tile(
                            [P, FOUT], f32, tag=f"acc{t % 2}", name=f"acc{t}"
                        )
                        for t in pair
                    }
                    for bk, t, sl, st, sp in mms:
                        if t not in pair:
                            continue
                        g_t, b_t = tiles[(bk, t)]
                        nc.tensor.matmul(
                            out=psum_t[t][:],
                            lhsT=b_t[:, sl, :],
                            rhs=g_t[:, sl, :FOUT],
                            start=st,
                            stop=sp,
                        )
                    # scale by dinv[dst] on the otherwise-idle Scalar engine
                    for t in pair:
                        if t >= nt:
                            continue
                        w = sw * prm.TPSW + t
                        nc.scalar.activation(
                            out=ysb[:, t, :],
                            in_=psum_t[t][:],
                            func=mybir.ActivationFunctionType.Copy,
                            scale=dinvD_sb[:, w : w + 1],
                        )
                for t in range(nt):
                    rt = min(P, rows_sw - t * P)
                    r0 = sw * prm.SWD + t * P
                    nc.sync.dma_start(out=y[r0 : r0 + rt, :], in_=ysb[:rt, t, :])

            for sw in range(prm.NSW):
                tiles = {}
                for bk in range(prm.NBK):
                    emit_calls(sw, bk, tiles)
                emit_mms(sw, tiles)
            _stk.close()

    nc.compile()
    _split_sync_waits(nc)
    return nc


_CACHE = {}


def _get_program_and_prep(x, edge_index, W, prm):
    inputs, calls, mms_by_sw = _host_prep(x, edge_index, W, prm)
    icols = sum(8 * cm.S for cm in calls)
    scols = sum(cm.S for cm in calls)
    nc = _build_program(prm, calls, mms_by_sw, icols, scols)
    return nc, inputs


def kernel(x, edge_index, W):
    prm = Prm(N=int(x.shape[0]))
    nc, inputs = _get_program_and_prep(x, edge_index, W, prm)
    res = run_bass_kernel_spmd(nc, inputs, list(range(prm.C)))
    y = np.concatenate([res.results[c]["y"] for c in range(prm.C)], axis=0)
    return y.astype(np.float32)


def run_with_trace(x, edge_index, W, trace_cores=None):
    """test.py helper: returns (y, BassKernelResults) with profiling."""
    prm = Prm(N=int(x.shape[0]))
    nc, inputs = _get_program_and_prep(x, edge_index, W, prm)
    res = run_bass_kernel_spmd(
        nc, inputs, list(range(prm.C)), trace=True, trace_cores=trace_cores
    )
    y = np.concatenate([res.results[c]["y"] for c in range(prm.C)], axis=0)
    return y.astype(np.float32), res
